# revision 29
# baseline (speedup 1.0000x reference)
"""Trainium2 Bass kernel for LocalSelfAttentionUnFold — band-sum factorized.

Reference math (B=4, S=2048, E=256, H=8, D=32, W=33, pad=16, K=S-W+1=2016):
  q,k,v = x @ W* + b*    -> heads [B,H,S,D];  q pre-scaled by D^-0.5
  scores[s,kx] = sum_{w<33} q_pad[s+w]·k[kx+w]      (dense [S,K] softmax over kx)
  out = softmax(scores) @ vsum,  vsum[kx] = sum_w v[kx+w]

Key identity: scores = D11 + sigma11(D11) + sigma22(D11) where
  D11[kx,s] = sum_{w<11} q_pad[s+w]·k[kx+w]   (computed TRANSPOSED: kx on partitions)
and sigma_d(X)[kx,s] = X[kx+d, s+d].  Post-exp this becomes a 3-factor
elementwise product: exp(scores) = A ⊙ sigma11(A) ⊙ sigma22(A), A = exp(D11).
So the PE does only a w-window of 11 (3 matmul passes instead of 8.25);
ACT does one exp pass (bf16 out — range covers e^38..e^-31, no max pass);
the diagonal-shifted copies are SBUF->SBUF DMAs; DVE does 2 bf16 muls.
Scores transposed => no attn transpose: AV matmul takes A33 tiles as lhsT
directly, with a ones-column appended to vsum so row-sums come free.
Normalization (divide by rowsum) happens on host.

Per core (8 cores): batch b=c//2, head group hg=c%2 (4 heads = 128 cols).
"""

import numpy as np
from contextlib import ExitStack

S = 2048
E = 256
D = 32
WIN = 33
PAD = 16
K = S - WIN + 1  # 2016
NHPC = 4  # heads per core
SCALE = float(D) ** -0.5
NCORES = 8
SE = S + 22   # 2070: extended s range (col shifts up to +22)
NT = 20       # kx tiles, 128 rows each, stride ST (overlap 27 so the
ST = 101      # sigma11/sigma22 shifted reads stay within one tile)

_CACHE: dict = {}


def _build_nc(reps=1):
    import concourse.bass as bass
    import concourse.tile as tile
    from concourse import bacc, mybir

    fp16 = mybir.dt.float16
    bf16 = mybir.dt.bfloat16
    f32 = mybir.dt.float32
    AF = mybir.ActivationFunctionType

    nc = bacc.Bacc("TRN2", target_bir_lowering=False, debug=False,
                   num_devices=NCORES)

    xT_d = nc.dram_tensor("xT", [E, S], f32, kind="ExternalInput").ap()
    wq_d = nc.dram_tensor("wq", [E, 128], f32, kind="ExternalInput").ap()
    wk_d = nc.dram_tensor("wk", [E, 128], f32, kind="ExternalInput").ap()
    bqs_d = nc.dram_tensor("bqs", [128, 1], f32, kind="ExternalInput").ap()
    bk_d = nc.dram_tensor("bk", [128, 1], f32, kind="ExternalInput").ap()
    bk4_d = nc.dram_tensor("bk4", [128, 1], f32, kind="ExternalInput").ap()
    vsaug_d = nc.dram_tensor("vsaug", [NHPC, 128, NT, 33], bf16,
                             kind="ExternalInput").ap()
    # raw AV output: per head 33 cols (32 out dims + rowsum); host divides
    po_d = nc.dram_tensor("po", [S, NHPC * 33], f32, kind="ExternalOutput").ap()

    with tile.TileContext(nc) as tc, ExitStack() as ctx:
        const = ctx.enter_context(tc.tile_pool(name="const", bufs=1))
        persist = ctx.enter_context(tc.tile_pool(name="persist", bufs=1))

        # ---- load inputs (gpsimd DMAs cast f32 -> fp16 in flight) ----
        x16 = persist.tile([128, 2, S], fp16)
        w16 = {}
        biases = {}
        for name, wd in (("k", wk_d), ("q", wq_d)):
            wt = const.tile([128, 2, 128], fp16, tag=f"w{name}")
            wf = const.tile([128, 2, 128], f32, tag=f"wf{name}")
            for i in range(2):
                nc.scalar.dma_start(out=wf[:, i, :], in_=wd[i * 128:(i + 1) * 128, :])
                nc.vector.tensor_copy(out=wt[:, i, :], in_=wf[:, i, :])
            w16[name] = wt
        for name, bd in (("k", bk_d), ("q", bqs_d), ("k4", bk4_d)):
            bt = const.tile([128, 1], f32, tag=f"b{name}")
            nc.scalar.dma_start(out=bt[:], in_=bd[:, :])
            biases[name] = bt
        for sb in range(4):
            for i in range(2):
                # cast f32->fp16 in flight: SWDGE (gpsimd) only
                nc.gpsimd.dma_start(
                    out=x16[:, i, sb * 512:(sb + 1) * 512],
                    in_=xT_d[i * 128:(i + 1) * 128, sb * 512:(sb + 1) * 512])

        # ---- projections: q^T,k^T,v^T [128, S] fp16 (q pre-scaled) ----
        qkv16 = {}
        with tc.tile_pool(name="pproj", bufs=2, space="PSUM") as pproj:
            for name in ("k", "q"):
                dst = persist.tile([128, S], fp16, tag=f"{name}16T")
                qkv16[name] = dst
                sc = SCALE if name == "q" else 1.0
                for sb in range(4):
                    ps = pproj.tile([128, 512], f32, tag="pp")
                    nc.tensor.matmul(ps[:], lhsT=w16[name][:, 0, :],
                                     rhs=x16[:, 0, sb * 512:(sb + 1) * 512],
                                     start=True, stop=False)
                    nc.tensor.matmul(ps[:], lhsT=w16[name][:, 1, :],
                                     rhs=x16[:, 1, sb * 512:(sb + 1) * 512],
                                     start=False, stop=True)
                    nc.scalar.activation(out=dst[:, sb * 512:(sb + 1) * 512],
                                         in_=ps[:], func=AF.Identity,
                                         bias=biases[name], scale=sc)
        q16T, k16T = qkv16["q"], qkv16["k"]

        # ---- SBUF pools ----
        kq = ctx.enter_context(tc.tile_pool(name="kq", bufs=2))
        vap = ctx.enter_context(tc.tile_pool(name="vap", bufs=2))
        a11p = ctx.enter_context(tc.tile_pool(name="a11p", bufs=4))
        s1p = ctx.enter_context(tc.tile_pool(name="s1p", bufs=3))
        s2p = ctx.enter_context(tc.tile_pool(name="s2p", bufs=3))
        p1p = ctx.enter_context(tc.tile_pool(name="p1p", bufs=3))
        a33p = ctx.enter_context(tc.tile_pool(name="a33p", bufs=1))
        poev = ctx.enter_context(tc.tile_pool(name="poev", bufs=2))

        # ---- head 0's K4s built straight from projection-style matmuls
        # (keeps the PE busy during setup instead of waiting on DMA builds);
        # Q4s comes via DMA from q16T in parallel.
        k4s0 = kq.tile([128, 2058], fp16, tag="k4s")
        q4s0 = kq.tile([128, 2080], fp16, tag="q4s")
        for r in range(4):
            nc.vector.memset(k4s0[32 * r:32 * r + 32, 2048 - r:2058], 0.0)
            nc.vector.memset(q4s0[32 * r:32 * r + 32, 0:16 - r], 0.0)
            nc.vector.memset(q4s0[32 * r:32 * r + 32, 2064 - r:2080], 0.0)
            nc.sync.dma_start(out=q4s0[32 * r:32 * r + 32, 16 - r:2064 - r],
                              in_=q16T[0:32, 0:2048])
        with tc.tile_pool(name="pdir", bufs=2, space="PSUM") as pdir:
            for sb in range(4):
                ps = pdir.tile([128, 512], f32, tag="pd")
                for r in range(4):
                    w = 512 if sb < 3 else 512 - r
                    for i in range(2):
                        nc.tensor.matmul(
                            ps[32 * r:32 * r + 32, 0:w],
                            lhsT=w16["k"][:, i, 0:32],
                            rhs=x16[:, i, sb * 512 + r:sb * 512 + r + w],
                            start=(i == 0), stop=(i == 1),
                            tile_position=(0, 32 * r))
                if sb < 3:
                    nc.vector.tensor_scalar_add(
                        k4s0[:, sb * 512:(sb + 1) * 512], ps[:],
                        biases["k4"][:])
                else:
                    for r in range(4):
                        w = 512 - r
                        nc.vector.tensor_scalar_add(
                            k4s0[32 * r:32 * r + 32, sb * 512:sb * 512 + w],
                            ps[32 * r:32 * r + 32, 0:w],
                            biases["k4"][32 * r:32 * r + 32])

        # ---- PSUM pools for the main loop (after setup PSUM released) ----
        pap = ctx.enter_context(tc.tile_pool(name="pap", bufs=1, space="PSUM"))
        pbp = ctx.enter_context(tc.tile_pool(name="pbp", bufs=1, space="PSUM"))
        pop = ctx.enter_context(tc.tile_pool(name="pop", bufs=2, space="PSUM"))

        po_r = po_d.rearrange("(sc p) (hh j) -> p sc hh j", p=128, hh=NHPC)

        def setup_head(h, prebuilt=None):
            """Build K4s/Q4s shifted operand tiles + vs_aug for head h."""
            hp = 32 * h
            if prebuilt is not None:
                K4s, Q4s = prebuilt
            else:
                # K4s[32r+d, j] = k[j+r, hp+d]; zero j >= 2048-r (to 2058)
                K4s = kq.tile([128, 2058], fp16, tag="k4s")
                for r in range(4):
                    nc.vector.memset(K4s[32 * r:32 * r + 32, 2048 - r:2058], 0.0)
                    nc.gpsimd.dma_start(out=K4s[32 * r:32 * r + 32, 0:2048 - r],
                                        in_=k16T[hp:hp + 32, r:2048])
                # Q4s[32r+d, i] = q_pad[i+r, hp+d] = q[i+r-16]; zeros outside
                Q4s = kq.tile([128, 2080], fp16, tag="q4s")
                for r in range(4):
                    nc.vector.memset(Q4s[32 * r:32 * r + 32, 0:16 - r], 0.0)
                    nc.vector.memset(Q4s[32 * r:32 * r + 32, 2064 - r:2080], 0.0)
                    nc.sync.dma_start(out=Q4s[32 * r:32 * r + 32, 16 - r:2064 - r],
                                      in_=q16T[hp:hp + 32, 0:2048])
            # vs_aug[p, t, 0:32] = vsum[ST*t+p], col 32 = ones mask
            # (host-precomputed, zero-padded past kx >= K)
            vs_aug = vap.tile([128, NT, 33], bf16, tag="vsaug")
            nc.sync.dma_start(out=vs_aug[:], in_=vsaug_d[h % NHPC])
            # A33[:, t, :] = attn^T (unnorm) for kx rows [ST*t, ST*t+101)
            A33 = a33p.tile([128, NT, S], bf16, tag="a33")
            return K4s, Q4s, vs_aug, A33

        def tile_job(st8, t):
            """Scores D11 -> exp -> shifted copies -> muls for kx-tile t."""
            K4s, Q4s, vs_aug, A33, a11 = st8
            kx0 = ST * t
            Pa = pap.tile([128, 1024], f32, tag="pa")
            Pb = pbp.tile([128, 1046], f32, tag="pb")
            # all Pa matmuls first, then Pb: with single-buffered PSUM the
            # next tile's Pa work overlaps exp(Pb) of this tile
            for oi, off in ((0, 0), (1, 4), (2, 8)):
                rows = 96 if oi == 2 else 128
                st, sp = (oi == 0), (oi == 2)
                lhs = K4s[0:rows, kx0 + off:kx0 + off + 128]
                nc.tensor.matmul(Pa[:, 0:512], lhsT=lhs,
                                 rhs=Q4s[0:rows, off:off + 512],
                                 start=st, stop=sp)
                nc.tensor.matmul(Pa[:, 512:1024], lhsT=lhs,
                                 rhs=Q4s[0:rows, 512 + off:1024 + off],
                                 start=st, stop=sp)
            for oi, off in ((0, 0), (1, 4), (2, 8)):
                rows = 96 if oi == 2 else 128
                st, sp = (oi == 0), (oi == 2)
                lhs = K4s[0:rows, kx0 + off:kx0 + off + 128]
                nc.tensor.matmul(Pb[:, 0:512], lhsT=lhs,
                                 rhs=Q4s[0:rows, 1024 + off:1536 + off],
                                 start=st, stop=sp)
                nc.tensor.matmul(Pb[:, 512:1024], lhsT=lhs,
                                 rhs=Q4s[0:rows, 1536 + off:2048 + off],
                                 start=st, stop=sp)
                nc.tensor.matmul(Pb[:, 1024:1046], lhsT=lhs,
                                 rhs=Q4s[0:rows, 2048 + off:SE + off],
                                 start=st, stop=sp)
            At = a11p.tile([128, SE], bf16, tag="a11")
            a11[t] = At
            nc.scalar.activation(out=At[:, 0:1024], in_=Pa[:],
                                 func=AF.Exp, bias=0.0, scale=1.0)
            nc.scalar.activation(out=At[:, 1024:SE], in_=Pb[:],
                                 func=AF.Exp, bias=0.0, scale=1.0)
            # combine: A33[.,t,.] = At * sigma11(At) * sigma22(At)
            S1 = s1p.tile([101, S], bf16, tag="s1")
            nc.sync.dma_start(out=S1[0:101, :], in_=At[11:112, 11:11 + S])
            S2 = s2p.tile([101, S], bf16, tag="s2")
            nc.gpsimd.dma_start(out=S2[0:101, :], in_=At[22:123, 22:22 + S])
            P1 = p1p.tile([101, S], bf16, tag="p1")
            nc.vector.tensor_mul(P1[0:101, :], At[0:101, 0:S], S1[0:101, :])
            # mul2 (the A33 write) is DEFERRED by the caller: the previous
            # head's AV reads of this same single-buffered A33 memory must
            # be emitted first (WAR on pool reuse)
            return (A33, t, P1, S2)

        poeh_of = {}

        def av_sc(h, sc, A33, vs_aug):
            """One s-chunk of head h's AV: 20 accumulating matmuls."""
            if sc == 0:
                poeh_of[h] = poev.tile([128, 16, 33], f32, tag="poeh",
                                       name="poeh")
            po = pop.tile([128, 33], f32, tag="po", name="po")
            for t in range(NT):
                nc.tensor.matmul(po[:],
                                 lhsT=A33[0:101, t, sc * 128:(sc + 1) * 128],
                                 rhs=vs_aug[0:101, t, :],
                                 start=(t == 0), stop=(t == NT - 1))
            poeh = poeh_of[h]
            nc.vector.tensor_copy(out=poeh[:, sc, :], in_=po[:])
            hh = h % NHPC
            nc.gpsimd.dma_start(
                out=po_d[sc * 128:(sc + 1) * 128, hh * 33:(hh + 1) * 33],
                in_=poeh[:, sc, :])
            if sc == 15:
                del poeh_of[h]

        # AV work for a finished head is spread across the next head's
        # first tile jobs so the PE never drains at head boundaries.  The
        # A33 writes (mul2) of tiles 0..3 are deferred until after that AV
        # drain: A33 is single-buffered, so the previous head's AV readers
        # must be emitted before the next head's first writers.
        av_tasks = []
        mul2q = []

        def flush_mul2():
            while mul2q:
                A33w, tw, P1w, S2w = mul2q.pop(0)
                nc.vector.tensor_mul(A33w[0:101, tw, :], P1w[0:101, :],
                                     S2w[0:101, :])

        pre = (k4s0, q4s0)
        nxt = None
        for rep in range(reps):
            for h in range(NHPC):
                if nxt is None:
                    nxt = setup_head(h, prebuilt=pre)
                st8 = (*nxt, {})
                K4s, Q4s, vs_aug, A33 = nxt
                nxt = None
                for t in range(NT):
                    mul2q.append(tile_job(st8, t))
                    if t == 12 and not (rep == reps - 1 and h == NHPC - 1):
                        nxt = setup_head((h + 1) % NHPC)
                    for _ in range(min(4, len(av_tasks))):
                        av_sc(*av_tasks.pop(0))
                    if t >= 4:
                        flush_mul2()
                av_tasks += [(rep * NHPC + h, sc, A33, vs_aug)
                             for sc in range(16)]
        flush_mul2()
        while av_tasks:
            av_sc(*av_tasks.pop(0))

    nc.compile()
    return nc


def _get_nc():
    if "nc" not in _CACHE:
        _CACHE["nc"] = _build_nc()
    return _CACHE["nc"]


def build_in_maps(x, Wq, bq, Wk, bk, Wv, bv):
    import ml_dtypes

    bfd = ml_dtypes.bfloat16
    x = np.asarray(x, dtype=np.float32)
    # host-side v projection + box-filter vsum (0.4% of total FLOPs):
    # vsaug[c][h, p, t, 0:32] = vsum[ST*t+p] of head h, col 32 = ones mask
    v = x @ np.asarray(Wv, np.float32) + np.asarray(bv, np.float32)  # [4,S,E]
    cs = np.zeros((4, S + 1, E), np.float32)
    cs[:, 1:] = np.cumsum(v, axis=1)
    vsum = cs[:, WIN:S + 1] - cs[:, 0:K]  # [4, K, E]
    in_maps = []
    for c in range(NCORES):
        b, hg = c // 2, c % 2
        sl = slice(hg * 128, (hg + 1) * 128)
        va = np.zeros((NHPC, 128, NT, 33), np.float32)
        idx = ST * np.arange(NT)[None, :] + np.arange(128)[:, None]  # [128,NT]
        valid = idx < K
        idxc = np.minimum(idx, K - 1)
        for h in range(NHPC):
            vh = vsum[b][:, hg * 128 + h * 32: hg * 128 + (h + 1) * 32]
            va[h, :, :, 0:32] = vh[idxc] * valid[:, :, None]
            va[h, :, :, 32] = valid.astype(np.float32)
        in_maps.append({
            "xT": np.ascontiguousarray(x[b].T),
            "wq": np.ascontiguousarray(np.asarray(Wq, np.float32)[:, sl]),
            "wk": np.ascontiguousarray(np.asarray(Wk, np.float32)[:, sl]),
            "bqs": np.ascontiguousarray(
                (np.asarray(bq, np.float32)[sl] * SCALE).reshape(128, 1)),
            "bk": np.ascontiguousarray(np.asarray(bk, np.float32)[sl].reshape(128, 1)),
            "bk4": np.ascontiguousarray(np.tile(
                np.asarray(bk, np.float32)[sl][0:32], 4).reshape(128, 1)),
            "vsaug": np.ascontiguousarray(va.astype(bfd)),
        })
    return in_maps


def kernel(x, Wq, bq, Wk, bk, Wv, bv):
    from concourse.bass_utils import run_bass_kernel_spmd

    nc = _get_nc()
    in_maps = build_in_maps(x, Wq, bq, Wk, bk, Wv, bv)
    res = run_bass_kernel_spmd(nc, in_maps, list(range(NCORES)))
    out = np.empty((4, S, E), np.float32)
    for c in range(NCORES):
        b, hg = c // 2, c % 2
        po = res.results[c]["po"]  # [S, NHPC*33]
        for h in range(NHPC):
            blk = po[:, h * 33:(h + 1) * 33]
            out[b, :, hg * 128 + h * 32: hg * 128 + (h + 1) * 32] = (
                blk[:, 0:32] / blk[:, 32:33])
    return out


# revision 36
# speedup vs baseline: 1.3312x; 1.3312x over previous
"""Trainium2 Bass kernel for LocalSelfAttentionUnFold — band-sum factorized.

Reference math (B=4, S=2048, E=256, H=8, D=32, W=33, pad=16, K=S-W+1=2016):
  q,k,v = x @ W* + b*    -> heads [B,H,S,D];  q pre-scaled by D^-0.5
  scores[s,kx] = sum_{w<33} q_pad[s+w]·k[kx+w]      (dense [S,K] softmax over kx)
  out = softmax(scores) @ vsum,  vsum[kx] = sum_w v[kx+w]

Key identity: scores = D11 + sigma11(D11) + sigma22(D11) where
  D11[kx,s] = sum_{w<11} q_pad[s+w]·k[kx+w]   (computed TRANSPOSED: kx on partitions)
and sigma_d(X)[kx,s] = X[kx+d, s+d].  Post-exp this becomes a 3-factor
elementwise product: exp(scores) = A ⊙ sigma11(A) ⊙ sigma22(A), A = exp(D11).
So the PE does only a w-window of 11 (3 matmul passes instead of 8.25);
ACT does one exp pass (bf16 out — range covers e^38..e^-31, no max pass);
the diagonal-shifted copies are SBUF->SBUF DMAs; DVE does 2 bf16 muls.
Scores transposed => no attn transpose: AV matmul takes A33 tiles as lhsT
directly, with a ones-column appended to vsum so row-sums come free.
Normalization (divide by rowsum) happens on host.

Per core (8 cores): batch b=c//2, head group hg=c%2 (4 heads = 128 cols).
"""

import numpy as np
from contextlib import ExitStack

S = 2048
E = 256
D = 32
WIN = 33
PAD = 16
K = S - WIN + 1  # 2016
NHPC = 4  # heads per core
SCALE = float(D) ** -0.5
NCORES = 8
SE = S + 22   # 2070: extended s range (col shifts up to +22)
NT = 20       # kx tiles, 128 rows each, stride ST (overlap 27 so the
ST = 101      # sigma11/sigma22 shifted reads stay within one tile)

_CACHE: dict = {}


def _build_nc(reps=1, no_sigma=False, no_mul=False, no_av=False,
              no_scores=False):
    import concourse.bass as bass
    import concourse.tile as tile
    from concourse import bacc, mybir

    fp16 = mybir.dt.float16
    bf16 = mybir.dt.bfloat16
    f32 = mybir.dt.float32
    AF = mybir.ActivationFunctionType

    nc = bacc.Bacc("TRN2", target_bir_lowering=False, debug=False,
                   num_devices=NCORES)

    xT_d = nc.dram_tensor("xT", [E, S], f32, kind="ExternalInput").ap()
    wq_d = nc.dram_tensor("wq", [E, 128], f32, kind="ExternalInput").ap()
    wk_d = nc.dram_tensor("wk", [E, 128], f32, kind="ExternalInput").ap()
    bqs_d = nc.dram_tensor("bqs", [128, 1], f32, kind="ExternalInput").ap()
    bk_d = nc.dram_tensor("bk", [128, 1], f32, kind="ExternalInput").ap()
    bk4_d = nc.dram_tensor("bk4", [128, 1], f32, kind="ExternalInput").ap()
    vsaug_d = nc.dram_tensor("vsaug", [NHPC, 128, NT, 33], bf16,
                             kind="ExternalInput").ap()
    # raw AV output: per head 33 cols (32 out dims + rowsum); host divides
    po_d = nc.dram_tensor("po", [S, NHPC * 33], f32, kind="ExternalOutput").ap()

    with tile.TileContext(nc) as tc, ExitStack() as ctx:
        const = ctx.enter_context(tc.tile_pool(name="const", bufs=1))
        persist = ctx.enter_context(tc.tile_pool(name="persist", bufs=1))

        # ---- load inputs (gpsimd DMAs cast f32 -> fp16 in flight) ----
        x16 = persist.tile([128, 2, S], fp16)
        w16 = {}
        biases = {}
        for name, wd in (("k", wk_d), ("q", wq_d)):
            wt = const.tile([128, 2, 128], fp16, tag=f"w{name}")
            wf = const.tile([128, 2, 128], f32, tag=f"wf{name}")
            for i in range(2):
                nc.scalar.dma_start(out=wf[:, i, :], in_=wd[i * 128:(i + 1) * 128, :])
                nc.vector.tensor_copy(out=wt[:, i, :], in_=wf[:, i, :])
            w16[name] = wt
        for name, bd in (("k", bk_d), ("q", bqs_d), ("k4", bk4_d)):
            bt = const.tile([128, 1], f32, tag=f"b{name}")
            nc.scalar.dma_start(out=bt[:], in_=bd[:, :])
            biases[name] = bt
        for sb in range(4):
            for i in range(2):
                # cast f32->fp16 in flight: SWDGE (gpsimd) only
                nc.gpsimd.dma_start(
                    out=x16[:, i, sb * 512:(sb + 1) * 512],
                    in_=xT_d[i * 128:(i + 1) * 128, sb * 512:(sb + 1) * 512])

        # ---- projections: q^T,k^T,v^T [128, S] fp16 (q pre-scaled) ----
        qkv16 = {}
        with tc.tile_pool(name="pproj", bufs=2, space="PSUM") as pproj:
            for name in ("k", "q"):
                dst = persist.tile([128, S], fp16, tag=f"{name}16T")
                qkv16[name] = dst
                sc = SCALE if name == "q" else 1.0
                for sb in range(4):
                    ps = pproj.tile([128, 512], f32, tag="pp")
                    nc.tensor.matmul(ps[:], lhsT=w16[name][:, 0, :],
                                     rhs=x16[:, 0, sb * 512:(sb + 1) * 512],
                                     start=True, stop=False)
                    nc.tensor.matmul(ps[:], lhsT=w16[name][:, 1, :],
                                     rhs=x16[:, 1, sb * 512:(sb + 1) * 512],
                                     start=False, stop=True)
                    nc.scalar.activation(out=dst[:, sb * 512:(sb + 1) * 512],
                                         in_=ps[:], func=AF.Identity,
                                         bias=biases[name], scale=sc)
        q16T, k16T = qkv16["q"], qkv16["k"]

        # ---- SBUF pools ----
        kq = ctx.enter_context(tc.tile_pool(name="kq", bufs=2))
        vap = ctx.enter_context(tc.tile_pool(name="vap", bufs=2))
        a11p = ctx.enter_context(tc.tile_pool(name="a11p", bufs=4))
        s1p = ctx.enter_context(tc.tile_pool(name="s1p", bufs=3))
        s2p = ctx.enter_context(tc.tile_pool(name="s2p", bufs=3))
        p1p = ctx.enter_context(tc.tile_pool(name="p1p", bufs=3))
        a33p = ctx.enter_context(tc.tile_pool(name="a33p", bufs=1))
        poev = ctx.enter_context(tc.tile_pool(name="poev", bufs=2))

        # ---- head 0's K4s built straight from projection-style matmuls
        # (keeps the PE busy during setup instead of waiting on DMA builds);
        # Q4s comes via DMA from q16T in parallel.
        k4s0 = kq.tile([128, 2058], fp16, tag="k4s")
        q4s0 = kq.tile([128, 2080], fp16, tag="q4s")
        for r in range(4):
            nc.vector.memset(k4s0[32 * r:32 * r + 32, 2048 - r:2058], 0.0)
            nc.vector.memset(q4s0[32 * r:32 * r + 32, 0:16 - r], 0.0)
            nc.vector.memset(q4s0[32 * r:32 * r + 32, 2064 - r:2080], 0.0)
            nc.sync.dma_start(out=q4s0[32 * r:32 * r + 32, 16 - r:2064 - r],
                              in_=q16T[0:32, 0:2048])
        with tc.tile_pool(name="pdir", bufs=2, space="PSUM") as pdir:
            for sb in range(4):
                ps = pdir.tile([128, 512], f32, tag="pd")
                for r in range(4):
                    w = 512 if sb < 3 else 512 - r
                    for i in range(2):
                        nc.tensor.matmul(
                            ps[32 * r:32 * r + 32, 0:w],
                            lhsT=w16["k"][:, i, 0:32],
                            rhs=x16[:, i, sb * 512 + r:sb * 512 + r + w],
                            start=(i == 0), stop=(i == 1),
                            tile_position=(0, 32 * r))
                if sb < 3:
                    nc.vector.tensor_scalar_add(
                        k4s0[:, sb * 512:(sb + 1) * 512], ps[:],
                        biases["k4"][:])
                else:
                    for r in range(4):
                        w = 512 - r
                        nc.vector.tensor_scalar_add(
                            k4s0[32 * r:32 * r + 32, sb * 512:sb * 512 + w],
                            ps[32 * r:32 * r + 32, 0:w],
                            biases["k4"][32 * r:32 * r + 32])

        # ---- PSUM pools for the main loop (after setup PSUM released) ----
        pap = ctx.enter_context(tc.tile_pool(name="pap", bufs=1, space="PSUM"))
        pbp = ctx.enter_context(tc.tile_pool(name="pbp", bufs=1, space="PSUM"))
        pop = ctx.enter_context(tc.tile_pool(name="pop", bufs=2, space="PSUM"))

        po_r = po_d.rearrange("(sc p) (hh j) -> p sc hh j", p=128, hh=NHPC)

        def setup_head(h, prebuilt=None):
            """Build K4s/Q4s shifted operand tiles + vs_aug for head h."""
            hp = 32 * h
            if prebuilt is not None:
                K4s, Q4s = prebuilt
            else:
                # K4s[32r+d, j] = k[j+r, hp+d]; zero j >= 2048-r (to 2058)
                K4s = kq.tile([128, 2058], fp16, tag="k4s")
                for r in range(4):
                    nc.vector.memset(K4s[32 * r:32 * r + 32, 2048 - r:2058], 0.0)
                    nc.gpsimd.dma_start(out=K4s[32 * r:32 * r + 32, 0:2048 - r],
                                        in_=k16T[hp:hp + 32, r:2048])
                # Q4s[32r+d, i] = q_pad[i+r, hp+d] = q[i+r-16]; zeros outside
                Q4s = kq.tile([128, 2080], fp16, tag="q4s")
                for r in range(4):
                    nc.vector.memset(Q4s[32 * r:32 * r + 32, 0:16 - r], 0.0)
                    nc.vector.memset(Q4s[32 * r:32 * r + 32, 2064 - r:2080], 0.0)
                    nc.gpsimd.dma_start(out=Q4s[32 * r:32 * r + 32, 16 - r:2064 - r],
                                        in_=q16T[hp:hp + 32, 0:2048])
            # vs_aug[p, t, 0:32] = vsum[ST*t+p], col 32 = ones mask
            # (host-precomputed, zero-padded past kx >= K)
            vs_aug = vap.tile([128, NT, 33], bf16, tag="vsaug")
            nc.sync.dma_start(out=vs_aug[:], in_=vsaug_d[h % NHPC])
            # A33[:, t, :] = attn^T (unnorm) for kx rows [ST*t, ST*t+101)
            A33 = a33p.tile([128, NT, S], bf16, tag="a33")
            return K4s, Q4s, vs_aug, A33

        def tile_job(st8, t):
            """Scores D11 -> exp -> shifted copies -> muls for kx-tile t."""
            K4s, Q4s, vs_aug, A33, a11 = st8
            kx0 = ST * t
            Pa = pap.tile([128, 1024], f32, tag="pa")
            Pb = pbp.tile([128, 1046], f32, tag="pb")
            # all Pa matmuls first, then Pb: with single-buffered PSUM the
            # next tile's Pa work overlaps exp(Pb) of this tile
            shifts = ((0, 0),) if no_scores else ((0, 0), (1, 4), (2, 8))
            for oi, off in shifts:
                rows = 96 if oi == 2 else 128
                st = (oi == 0)
                sp = (oi == (len(shifts) - 1))
                lhs = K4s[0:rows, kx0 + off:kx0 + off + 128]
                nc.tensor.matmul(Pa[:, 0:512], lhsT=lhs,
                                 rhs=Q4s[0:rows, off:off + 512],
                                 start=st, stop=sp)
                nc.tensor.matmul(Pa[:, 512:1024], lhsT=lhs,
                                 rhs=Q4s[0:rows, 512 + off:1024 + off],
                                 start=st, stop=sp)
            for oi, off in shifts:
                rows = 96 if oi == 2 else 128
                st = (oi == 0)
                sp = (oi == (len(shifts) - 1))
                lhs = K4s[0:rows, kx0 + off:kx0 + off + 128]
                nc.tensor.matmul(Pb[:, 0:512], lhsT=lhs,
                                 rhs=Q4s[0:rows, 1024 + off:1536 + off],
                                 start=st, stop=sp)
                nc.tensor.matmul(Pb[:, 512:1024], lhsT=lhs,
                                 rhs=Q4s[0:rows, 1536 + off:2048 + off],
                                 start=st, stop=sp)
                nc.tensor.matmul(Pb[:, 1024:1046], lhsT=lhs,
                                 rhs=Q4s[0:rows, 2048 + off:SE + off],
                                 start=st, stop=sp)
            At = a11p.tile([128, SE], bf16, tag="a11")
            a11[t] = At
            nc.scalar.activation(out=At[:, 0:1024], in_=Pa[:],
                                 func=AF.Exp, bias=0.0, scale=1.0)
            nc.scalar.activation(out=At[:, 1024:SE], in_=Pb[:],
                                 func=AF.Exp, bias=0.0, scale=1.0)
            # combine: A33[.,t,.] = At * sigma11(At) * sigma22(At)
            S1 = s1p.tile([101, S], bf16, tag="s1")
            S2 = s2p.tile([101, S], bf16, tag="s2")
            if not no_sigma:
                nc.sync.dma_start(out=S1[0:101, :], in_=At[11:112, 11:11 + S])
                nc.gpsimd.dma_start(out=S2[0:101, :], in_=At[22:123, 22:22 + S])
            else:
                nc.vector.tensor_copy(out=S1[0:101, :], in_=At[0:101, 0:S])
                nc.vector.tensor_copy(out=S2[0:101, :], in_=At[0:101, 0:S])
            P1 = p1p.tile([101, S], bf16, tag="p1")
            if not no_mul:
                nc.vector.tensor_mul(P1[0:101, :], At[0:101, 0:S], S1[0:101, :])
            else:
                nc.vector.tensor_copy(out=P1[0:101, :], in_=At[0:101, 0:S])
            # mul2 (the A33 write) is DEFERRED by the caller: the previous
            # head's AV reads of this same single-buffered A33 memory must
            # be emitted first (WAR on pool reuse)
            return (A33, t, P1, S2)

        poeh_of = {}

        def av_sc(h, sc, A33, vs_aug):
            """One s-chunk of head h's AV: 20 accumulating matmuls."""
            if sc == 0:
                poeh_of[h] = poev.tile([128, 16, 33], f32, tag="poeh",
                                       name="poeh")
            po = pop.tile([128, 33], f32, tag="po", name="po")
            nav = 1 if no_av else NT
            for t in range(nav):
                nc.tensor.matmul(po[:],
                                 lhsT=A33[0:101, t, sc * 128:(sc + 1) * 128],
                                 rhs=vs_aug[0:101, t, :],
                                 start=(t == 0), stop=(t == nav - 1))
            poeh = poeh_of[h]
            nc.vector.tensor_copy(out=poeh[:, sc, :], in_=po[:])
            hh = h % NHPC
            nc.sync.dma_start(
                out=po_d[sc * 128:(sc + 1) * 128, hh * 33:(hh + 1) * 33],
                in_=poeh[:, sc, :])
            if sc == 15:
                del poeh_of[h]

        # AV work for a finished head is spread across the next head's
        # first tile jobs so the PE never drains at head boundaries.  The
        # A33 writes (mul2) of tiles 0..3 are deferred until after that AV
        # drain: A33 is single-buffered, so the previous head's AV readers
        # must be emitted before the next head's first writers.
        av_tasks = []
        mul2q = []

        def flush_mul2():
            while mul2q:
                A33w, tw, P1w, S2w = mul2q.pop(0)
                if not no_mul:
                    nc.vector.tensor_mul(A33w[0:101, tw, :], P1w[0:101, :],
                                         S2w[0:101, :])
                else:
                    nc.vector.tensor_copy(out=A33w[0:101, tw, :],
                                          in_=P1w[0:101, :])

        pre = (k4s0, q4s0)
        nxt = None
        for rep in range(reps):
            for h in range(NHPC):
                if nxt is None:
                    nxt = setup_head(h, prebuilt=pre)
                st8 = (*nxt, {})
                K4s, Q4s, vs_aug, A33 = nxt
                nxt = None
                for t in range(NT):
                    mul2q.append(tile_job(st8, t))
                    if t == 12 and not (rep == reps - 1 and h == NHPC - 1):
                        nxt = setup_head((h + 1) % NHPC)
                    for _ in range(min(4, len(av_tasks))):
                        av_sc(*av_tasks.pop(0))
                    if t >= 4:
                        flush_mul2()
                av_tasks += [(rep * NHPC + h, sc, A33, vs_aug)
                             for sc in range(16)]
        flush_mul2()
        while av_tasks:
            av_sc(*av_tasks.pop(0))

    nc.compile()
    return nc


def _get_nc():
    if "nc" not in _CACHE:
        _CACHE["nc"] = _build_nc()
    return _CACHE["nc"]


def build_in_maps(x, Wq, bq, Wk, bk, Wv, bv):
    import ml_dtypes

    bfd = ml_dtypes.bfloat16
    x = np.asarray(x, dtype=np.float32)
    # host-side v projection + box-filter vsum (0.4% of total FLOPs):
    # vsaug[c][h, p, t, 0:32] = vsum[ST*t+p] of head h, col 32 = ones mask
    v = x @ np.asarray(Wv, np.float32) + np.asarray(bv, np.float32)  # [4,S,E]
    cs = np.zeros((4, S + 1, E), np.float32)
    cs[:, 1:] = np.cumsum(v, axis=1)
    vsum = cs[:, WIN:S + 1] - cs[:, 0:K]  # [4, K, E]
    in_maps = []
    for c in range(NCORES):
        b, hg = c // 2, c % 2
        sl = slice(hg * 128, (hg + 1) * 128)
        va = np.zeros((NHPC, 128, NT, 33), np.float32)
        idx = ST * np.arange(NT)[None, :] + np.arange(128)[:, None]  # [128,NT]
        valid = idx < K
        idxc = np.minimum(idx, K - 1)
        for h in range(NHPC):
            vh = vsum[b][:, hg * 128 + h * 32: hg * 128 + (h + 1) * 32]
            va[h, :, :, 0:32] = vh[idxc] * valid[:, :, None]
            va[h, :, :, 32] = valid.astype(np.float32)
        in_maps.append({
            "xT": np.ascontiguousarray(x[b].T),
            "wq": np.ascontiguousarray(np.asarray(Wq, np.float32)[:, sl]),
            "wk": np.ascontiguousarray(np.asarray(Wk, np.float32)[:, sl]),
            "bqs": np.ascontiguousarray(
                (np.asarray(bq, np.float32)[sl] * SCALE).reshape(128, 1)),
            "bk": np.ascontiguousarray(np.asarray(bk, np.float32)[sl].reshape(128, 1)),
            "bk4": np.ascontiguousarray(np.tile(
                np.asarray(bk, np.float32)[sl][0:32], 4).reshape(128, 1)),
            "vsaug": np.ascontiguousarray(va.astype(bfd)),
        })
    return in_maps


def kernel(x, Wq, bq, Wk, bk, Wv, bv):
    from concourse.bass_utils import run_bass_kernel_spmd

    nc = _get_nc()
    in_maps = build_in_maps(x, Wq, bq, Wk, bk, Wv, bv)
    res = run_bass_kernel_spmd(nc, in_maps, list(range(NCORES)))
    out = np.empty((4, S, E), np.float32)
    for c in range(NCORES):
        b, hg = c // 2, c % 2
        po = res.results[c]["po"]  # [S, NHPC*33]
        for h in range(NHPC):
            blk = po[:, h * 33:(h + 1) * 33]
            out[b, :, hg * 128 + h * 32: hg * 128 + (h + 1) * 32] = (
                blk[:, 0:32] / blk[:, 32:33])
    return out
